# revision 1
# baseline (speedup 1.0000x reference)
"""Trainium2 Bass kernel: 6-layer decoder (masked self-attn + cross-attn + FFN).

Sharding (8 cores): 4 batch pairs x 2-way sequence-parallel.
Core r: batch r//2, half g=r%2. Global 512-token chunks: g=0 owns [c0,c3],
g=1 owns [c1,c2] (zigzag for causal load balance). Activations are stored
core-locally as [my 1024 tokens | peer 1024 tokens]; causal structure is
identical across cores (union schedule), with per-core differences expressed
purely as data (exp-bias columns and diagonal mask constants).
One AllGather per layer (within each pair) exchanges the layer output.

Layout: feature-major ("T") everywhere: actT[feat, tok]. Matmuls compute
outT = W(lhsT).T @ actT(rhs) with float32r; scores/AV run in bf16.
"""

import os

import numpy as np
import ml_dtypes

import concourse.bass as bass
import concourse.mybir as mybir
import concourse.tile as tile
from concourse import bacc
from concourse.bass import ts
from concourse.bass_utils import run_bass_kernel_spmd

L, B, S, D, H, DK, F = 6, 4, 2048, 512, 8, 64, 2048
P = 128
TCH = 512                 # token chunk = matmul free dim
HALF = S // 2             # tokens owned per core
KC = D // P               # 4 partition chunks of d_model
NFT = F // P              # 16 feature tiles of FFN hidden
NKT = S // P              # 16 k-tiles over full sequence
AVW = DK + 1              # V columns per head + ones column (softmax sum)
EPS = 1e-5
SCALE = 1.0 / float(np.sqrt(DK))
NEG = -1e9

f32 = mybir.dt.float32
f32r = mybir.dt.float32r
bf16 = mybir.dt.bfloat16
AF = mybir.ActivationFunctionType
ALU = mybir.AluOpType

NLAYERS = int(os.environ.get("KERNEL_NLAYERS", str(L)))
OPT_ACTSET = bool(int(os.environ.get("KOPT_ACTSET", "1")))
OPT_GMASK = bool(int(os.environ.get("KOPT_GMASK", "1")))
OPT_RECIP = bool(int(os.environ.get("KOPT_RECIP", "0")))
RG = [[0, 1], [2, 3], [4, 5], [6, 7]]

# Union causal schedule (identical on every core). Local k-tile order:
# 0-3 = my chunk j0, 4-7 = my chunk j1, 8-11 = peer j0, 12-15 = peer j1.
SA_KT = {0: [0, 1, 2, 3, 8, 9, 10, 11], 1: list(range(16))}
CA_KT = {0: list(range(16)), 1: list(range(16))}


def _diag_shape(j, kt):
    if j == 0 and kt < 4:
        return kt
    if j == 1 and 4 <= kt < 8:
        return kt - 4
    return None


def _bias_col(j, kt):
    if kt >= 8:
        return (kt - 8) if j == 0 else 4 + (kt - 8)
    return None


def _r(ap):
    return ap.bitcast(f32r)


def _single_act_set():
    # Force every ACT function onto natural_log_exp_and_others (it contains
    # Exp, Ln, Identity and Relu) so the compiled kernel has exactly one
    # ACT_TABLE_LOAD instead of thrashing between per-function sets.
    real = bacc.get_activation_tables

    def patched(arch):
        tabs = real(arch)
        return {name: (fns if name == "natural_log_exp_and_others" else set())
                for name, fns in tabs.items()}

    bacc.get_activation_tables = patched


if OPT_ACTSET:
    _single_act_set()


def build(ln_affine: bool, v_bias: bool):
    nc = bacc.Bacc(None, target_bir_lowering=False, num_devices=8)

    xT = nc.declare_dram_parameter("xT", [P, KC, S], f32r, isOutput=False)
    encT = nc.declare_dram_parameter("encT", [P, KC, S], f32r, isOutput=False)
    w_in = {}
    for pre in ("sa", "ca"):
        for nm in ("wq", "wk", "wv"):
            w_in[f"{pre}_{nm}"] = nc.declare_dram_parameter(f"{pre}_{nm}", [L, D, D], f32r, isOutput=False)
        w_in[f"{pre}_wo"] = nc.declare_dram_parameter(f"{pre}_wo", [L, D, D], bf16, isOutput=False)
        for nm in ("bq", "bk", "bv", "bo"):
            w_in[f"{pre}_{nm}"] = nc.declare_dram_parameter(f"{pre}_{nm}", [L, D], f32, isOutput=False)
    w_in["ff_w1"] = nc.declare_dram_parameter("ff_w1", [L, D, F], bf16, isOutput=False)
    w_in["ff_b1"] = nc.declare_dram_parameter("ff_b1", [L, F], f32, isOutput=False)
    w_in["ff_w2b"] = nc.declare_dram_parameter("ff_w2b", [L, F, D], bf16, isOutput=False)
    w_in["ff_b2"] = nc.declare_dram_parameter("ff_b2", [L, D], f32, isOutput=False)
    if ln_affine:
        for i in (1, 2, 3):
            w_in[f"ln{i}_g"] = nc.declare_dram_parameter(f"ln{i}_g", [L, D], f32, isOutput=False)
            w_in[f"ln{i}_b"] = nc.declare_dram_parameter(f"ln{i}_b", [L, D], f32, isOutput=False)
    ones_in = nc.declare_dram_parameter("ones", [P, P], f32r, isOutput=False)
    ident_in = nc.declare_dram_parameter("ident", [P, P], f32r, isOutput=False)
    dmask_in = nc.declare_dram_parameter("dmask", [P, 4, TCH], bf16, isOutput=False)
    pbias_in = nc.declare_dram_parameter("pbias", [P, 12], f32, isOutput=False)
    out_p = nc.declare_dram_parameter("out", [P, KC, HALF], f32, isOutput=True)

    with tile.TileContext(nc, num_cores=8) as tc:
        import contextlib

        gctx = contextlib.ExitStack()
        with gctx:
            persist = gctx.enter_context(tc.tile_pool(name="persist", bufs=1))
            psA = gctx.enter_context(tc.tile_pool(name="psA", bufs=3, space="PSUM"))
            psS = gctx.enter_context(tc.tile_pool(name="psS", bufs=3, space="PSUM"))
            psO = gctx.enter_context(tc.tile_pool(name="psO", bufs=2, space="PSUM"))
            dramp = gctx.enter_context(tc.tile_pool(name="dramp", bufs=2, space="DRAM"))

            hT = persist.tile([P, KC, S], f32r, name="hT")
            ones_sb = persist.tile([P, P], f32r, name="ones_sb")
            ident_sb = persist.tile([P, P], f32r, name="ident_sb")
            dmask_sb = persist.tile([P, 4, TCH], bf16, name="dmask_sb")
            pbias_sb = persist.tile([P, 12], f32, name="pbias_sb")
            zero_sb = persist.tile([P, 1], f32, name="zero_sb")
            eps_sb = persist.tile([P, 1], f32, name="eps_sb")
            nc.vector.memset(zero_sb, 0.0)
            nc.vector.memset(eps_sb, EPS)

            for kc in range(KC):
                nc.sync.dma_start(out=hT[:, kc, :], in_=xT[:, kc, :])
            nc.sync.dma_start(out=ones_sb, in_=ones_in[:, :])
            nc.sync.dma_start(out=ident_sb, in_=ident_in[:, :])
            nc.sync.dma_start(out=dmask_sb, in_=dmask_in[:, :, :])
            nc.sync.dma_start(out=pbias_sb, in_=pbias_in[:, :])

            pid = nc.sync.partition_id()
            peer = (pid + 1) % 2

            def load_w(pool, dram_t, l, cols, dt=f32r, tag="w", bufs=3):
                n = dram_t.shape[1] // P
                l = l % L
                t = pool.tile([P, n, cols], dt, tag=tag, bufs=bufs, name=tag)
                for kc in range(n):
                    nc.sync.dma_start(out=t[:, kc, :], in_=dram_t[l, kc * P:(kc + 1) * P, :])
                return t

            def load_b(pool, dram_t, l, tag):
                n = dram_t.shape[1] // P
                l = l % L
                t = pool.tile([P, n], f32, tag=tag, bufs=2, name=tag)
                nc.sync.dma_start(out=t, in_=dram_t[l].rearrange("(c p) -> p c", p=P))
                return t

            def copy_ps(dst, src_ps, bias_ap, eng):
                # psum -> sbuf copy with fused per-partition bias add (DVE;
                # ACT is the busier engine)
                nc.vector.tensor_scalar(dst, src_ps, bias_ap, None, ALU.add)

            def kv_proj(lp, src_getter, wk_sb, bk_sb, wv_sb, bv_sb, kT_t, vaug_t,
                        ts_list=(0, 1, 2, 3), memset_first=True):
                # K^T feature-major (bf16) over full 2048 tokens + V token-major
                # into per-head augmented layout [P, NKT, H*AVW] (bf16).
                if memset_first:
                    nc.vector.memset(vaug_t, 1.0)
                for t in ts_list:
                    src = src_getter(t)  # [P, KC, TCH]
                    for ft in range(KC):
                        k_ps = psA.tile([P, TCH], f32, tag="acc", name="k_ps")
                        for kc in range(KC):
                            nc.tensor.matmul(k_ps, wk_sb[:, kc, ft * P:(ft + 1) * P],
                                             src[:, kc, :], start=(kc == 0), stop=(kc == KC - 1))
                        copy_ps(kT_t[:, ft, t * TCH:(t + 1) * TCH], k_ps, bk_sb[:, ft:ft + 1], (t + ft) % 2)
                    for tl in range(4):
                        tt = t * 4 + tl
                        v_ps = psA.tile([P, D], f32, tag="acc", name="v_ps")
                        nmm = KC + (1 if v_bias else 0)
                        for kc in range(KC):
                            nc.tensor.matmul(v_ps, src[:, kc, tl * P:(tl + 1) * P],
                                             wv_sb[:, kc, :], start=(kc == 0),
                                             stop=(kc == nmm - 1))
                        if v_bias:
                            nc.tensor.matmul(v_ps, ones_sb[0:1, :], bv_sb, start=False, stop=True)
                        for h in range(H):
                            nc.vector.tensor_copy(out=vaug_t[:, tt, h * AVW:h * AVW + DK],
                                                  in_=v_ps[:, h * DK:(h + 1) * DK])

            def q_proj(lp, srcT, wq_sb, bq_sb, qT_t):
                for t in range(2):
                    for ft in range(KC):
                        q_ps = psA.tile([P, TCH], f32, tag="acc", name="q_ps")
                        for kc in range(KC):
                            nc.tensor.matmul(q_ps, wq_sb[:, kc, ft * P:(ft + 1) * P],
                                             srcT[:, kc, t * TCH:(t + 1) * TCH],
                                             start=(kc == 0), stop=(kc == KC - 1))
                        copy_ps(qT_t[:, ft, t * TCH:(t + 1) * TCH], q_ps, bq_sb[:, ft:ft + 1], (t + ft) % 2)

            def attention(lp, pp, kT_t, vaug_t, qT_t, oT_t, causal):
                kts_by_j = SA_KT if causal else CA_KT
                CH = 4  # k-tiles per pipeline chunk
                for h in range(H):
                    for j in range(2):
                        kts = kts_by_j[j]
                        chunks = [kts[i:i + CH] for i in range(0, len(kts), CH)]
                        o_ps = psO.tile([AVW, TCH], f32, tag="oacc", name="o_ps")
                        done = 0
                        for ci, ch in enumerate(chunks):
                            pt = pp.tile([P, CH, TCH], bf16, tag="pt", bufs=3, name="pt")
                            for idx, kt in enumerate(ch):
                                s_ps = psS.tile([P, TCH], f32, tag="sc", name="s_ps")
                                nc.tensor.matmul(
                                    s_ps,
                                    kT_t[(h % 2) * DK:(h % 2) * DK + DK, h // 2, kt * P:(kt + 1) * P],
                                    qT_t[(h % 2) * DK:(h % 2) * DK + DK, h // 2, j * TCH:(j + 1) * TCH],
                                    start=True, stop=True)
                                bias = zero_sb[:, 0:1]
                                if causal:
                                    c = _bias_col(j, kt)
                                    if c is not None:
                                        bias = pbias_sb[:, c:c + 1]
                                nc.scalar.activation(pt[:, idx, :], s_ps, AF.Exp, bias=bias, scale=SCALE)
                                if causal:
                                    dsh = _diag_shape(j, kt)
                                    if dsh is not None:
                                        eng = nc.gpsimd if OPT_GMASK else nc.vector
                                        eng.tensor_mul(pt[:, idx, :], pt[:, idx, :], dmask_sb[:, dsh, :])
                            for idx, kt in enumerate(ch):
                                nc.tensor.matmul(o_ps, vaug_t[:, kt, h * AVW:(h + 1) * AVW],
                                                 pt[:, idx, :], start=(done == 0),
                                                 stop=(done == len(kts) - 1))
                                done += 1
                        rb = lp.tile([DK, TCH], f32, tag="rb", bufs=2, name="rb")
                        if OPT_RECIP:
                            srow = lp.tile([AVW, TCH], f32, tag="srow", bufs=2, name="srow")
                            nc.vector.tensor_copy(out=srow[DK:AVW, :], in_=o_ps[DK:AVW, :])
                            nc.vector.reciprocal_approx_fast(out=srow[DK:AVW, :], in_=srow[DK:AVW, :])
                            sdr = dramp.tile([1, TCH], f32, tag="snorm", bufs=4, name="sdr")
                            nc.sync.dma_start(out=sdr, in_=srow[DK:AVW, :])
                            bsrc = bass.AP(tensor=sdr.tensor, offset=sdr.offset,
                                           ap=[[0, DK]] + list(sdr.ap)[1:])
                            nc.sync.dma_start(out=rb, in_=bsrc)
                        else:
                            srow = lp.tile([AVW, TCH], f32r, tag="srow", bufs=2, name="srow")
                            nc.vector.tensor_copy(out=srow[DK:AVW, :], in_=o_ps[DK:AVW, :])
                            nc.scalar.activation(srow[DK:AVW, :], srow[DK:AVW, :], AF.Ln, bias=zero_sb[DK:DK + 1, 0:1])
                            nc.scalar.activation(srow[DK:AVW, :], srow[DK:AVW, :], AF.Exp, scale=-1.0, bias=zero_sb[DK:DK + 1, 0:1])
                            r_ps = psS.tile([DK, TCH], f32, tag="sc", name="r_ps")
                            nc.tensor.matmul(r_ps, ones_sb[DK:DK + 1, 0:DK], srow[DK:AVW, :],
                                             start=True, stop=True)
                            nc.vector.tensor_copy(out=rb, in_=r_ps)
                        nc.vector.tensor_mul(
                            oT_t[(h % 2) * DK:(h % 2) * DK + DK, h // 2, j * TCH:(j + 1) * TCH],
                            o_ps[0:DK, :], rb)

            def out_proj(lp, oT_t, wo_sb, bo_sb, resT, u_t):
                # u = oT @ wo + bo + res   (res added via identity matmul)
                for ft in range(KC):
                    pss = [psA.tile([P, TCH], f32, tag="acc", name="u_ps") for _ in range(2)]
                    for kc in range(KC):
                        for t in range(2):
                            nc.tensor.matmul(pss[t], wo_sb[:, kc, ft * P:(ft + 1) * P],
                                             oT_t[:, kc, t * TCH:(t + 1) * TCH],
                                             start=(kc == 0), stop=False)
                    for t in range(2):
                        nc.tensor.matmul(pss[t], ident_sb, resT(ft, t), start=False, stop=True)
                        copy_ps(u_t[:, ft, t * TCH:(t + 1) * TCH], pss[t], bo_sb[:, ft:ft + 1], t % 2)

            def layernorm(lp, rows, u_t, dst, g_sb, b_sb):
                for t in range(2):
                    tsl = slice(t * TCH, (t + 1) * TCH)
                    usq = lp.tile([P, KC, TCH], f32r, tag="usq", bufs=1, name="usq")
                    for kc in range(KC):
                        nc.vector.tensor_mul(usq[:, kc, :], u_t[:, kc, tsl], u_t[:, kc, tsl])
                    m_ps = psS.tile([P, TCH], f32, tag="sc", name="m_ps")
                    for kc in range(KC):
                        nc.tensor.matmul(m_ps, ones_sb, u_t[:, kc, tsl],
                                         start=(kc == 0), stop=(kc == KC - 1))
                    q_ps = psS.tile([P, TCH], f32, tag="sc", name="q_ps")
                    for kc in range(KC):
                        nc.tensor.matmul(q_ps, ones_sb, usq[:, kc, :],
                                         start=(kc == 0), stop=(kc == KC - 1))
                    t_sb = rows.tile([P, TCH], f32, tag="rows", bufs=4, name="t_sb")
                    nc.vector.tensor_scalar(t_sb, m_ps, 1.0 / D, None, ALU.mult)
                    m2 = rows.tile([P, TCH], f32, tag="rows", bufs=4, name="m2")
                    nc.vector.tensor_scalar(m2, q_ps, 1.0 / D, None, ALU.mult)
                    tt2 = rows.tile([P, TCH], f32, tag="rows", bufs=4, name="tt2")
                    nc.vector.tensor_mul(tt2, t_sb, t_sb)
                    nc.vector.tensor_sub(m2, m2, tt2)
                    nc.scalar.activation(m2, m2, AF.Ln, bias=eps_sb[:, 0:1])
                    r_sb = rows.tile([P, TCH], f32, tag="rows", bufs=4, name="r_sb")
                    nc.scalar.activation(r_sb, m2, AF.Exp, scale=-0.5, bias=zero_sb[:, 0:1])
                    c_sb = rows.tile([P, TCH], f32, tag="rows", bufs=4, name="c_sb")
                    nc.vector.tensor_mul(c_sb, t_sb, r_sb)
                    for kc in range(KC):
                        tmp = rows.tile([P, TCH], f32, tag="ltmp", bufs=2, name="ltmp")
                        nc.vector.tensor_sub(tmp, u_t[:, kc, tsl], c_sb)
                        d = dst(kc, t)
                        nc.vector.tensor_mul(d, tmp, r_sb)
                        if ln_affine:
                            nc.vector.tensor_scalar(d, d, g_sb[:, kc:kc + 1], b_sb[:, kc:kc + 1],
                                                    ALU.mult, ALU.add)

            for l in range(NLAYERS):
                lctx = contextlib.ExitStack()
                with lctx:
                    lp = lctx.enter_context(tc.tile_pool(name=f"lay{l}", bufs=1))
                    rows = lctx.enter_context(tc.tile_pool(name=f"rows{l}", bufs=1))
                    x1T = lp.tile([P, KC, HALF], f32r, tag="x1", name="x1T")
                    yT = lp.tile([P, KC, HALF], bf16, tag="y", name="yT")

                    # ---- self-attention ----
                    sctx = contextlib.ExitStack()
                    with sctx:
                        sp = sctx.enter_context(tc.tile_pool(name=f"sa{l}", bufs=1))
                        pp = sctx.enter_context(tc.tile_pool(name=f"sapt{l}", bufs=1))
                        kT_t = sp.tile([P, KC, S], bf16, tag="kT", name="kT_sa")
                        vaug_t = sp.tile([P, NKT, H * AVW], bf16, tag="vaug", name="vaug_sa")
                        qT_t = sp.tile([P, KC, HALF], bf16, tag="qT", name="qT_sa")
                        oT_t = sp.tile([P, KC, HALF], bf16, tag="oT", name="oT_sa")
                        u1 = lp.tile([P, KC, HALF], f32r, tag="u", bufs=1, name="u1")

                        wk_sb = load_w(sp, w_in["sa_wk"], l, D)
                        wv_sb = load_w(sp, w_in["sa_wv"], l, D)
                        bk_sb = load_b(sp, w_in["sa_bk"], l, "bk")
                        bv_sb = None
                        if v_bias:
                            bv_sb = sp.tile([1, D], f32, tag="bv", bufs=2, name="bv")
                            nc.sync.dma_start(out=bv_sb, in_=w_in["sa_bv"][l % L:l % L + 1, :])
                        kv_proj(sp, lambda t: hT[:, :, t * TCH:(t + 1) * TCH],
                                wk_sb, bk_sb, wv_sb, bv_sb, kT_t, vaug_t, ts_list=(0, 1))
                        wq_sb = load_w(sp, w_in["sa_wq"], l, D)
                        bq_sb = load_b(sp, w_in["sa_bq"], l, "bq")
                        q_proj(sp, hT[:, :, 0:HALF], wq_sb, bq_sb, qT_t)
                        kv_proj(sp, lambda t: hT[:, :, t * TCH:(t + 1) * TCH],
                                wk_sb, bk_sb, wv_sb, bv_sb, kT_t, vaug_t,
                                ts_list=(2, 3), memset_first=False)
                        attention(sp, pp, kT_t, vaug_t, qT_t, oT_t, causal=True)
                        wo_sb = load_w(sp, w_in["sa_wo"], l, D, dt=bf16)
                        bo_sb = load_b(sp, w_in["sa_bo"], l, "bo")
                        out_proj(sp, oT_t, wo_sb, bo_sb,
                                 lambda ft, t: hT[:, ft, t * TCH:(t + 1) * TCH], u1)
                        g1 = load_b(sp, w_in["ln1_g"], l, "g1") if ln_affine else None
                        b1l = load_b(sp, w_in["ln1_b"], l, "b1l") if ln_affine else None
                        layernorm(lp, rows, u1, lambda kc, t: x1T[:, kc, t * TCH:(t + 1) * TCH], g1, b1l)

                    # ---- cross-attention ----
                    cctx = contextlib.ExitStack()
                    with cctx:
                        cp = cctx.enter_context(tc.tile_pool(name=f"ca{l}", bufs=1))
                        pp2 = cctx.enter_context(tc.tile_pool(name=f"capt{l}", bufs=1))
                        kT_t = cp.tile([P, KC, S], bf16, tag="kT", name="kT_ca")
                        vaug_t = cp.tile([P, NKT, H * AVW], bf16, tag="vaug", name="vaug_ca")
                        qT_t = cp.tile([P, KC, HALF], bf16, tag="qT", name="qT_ca")
                        oT_t = cp.tile([P, KC, HALF], bf16, tag="oT", name="oT_ca")
                        u2 = lp.tile([P, KC, HALF], f32r, tag="u", bufs=1, name="u2")

                        def enc_chunk(t):
                            ec = cp.tile([P, KC, TCH], f32r, tag="enc", bufs=2, name="encC")
                            for kc in range(KC):
                                nc.sync.dma_start(out=ec[:, kc, :],
                                                  in_=encT[:, kc, t * TCH:(t + 1) * TCH])
                            return ec

                        wk_sb = load_w(cp, w_in["ca_wk"], l, D)
                        wv_sb = load_w(cp, w_in["ca_wv"], l, D)
                        bk_sb = load_b(cp, w_in["ca_bk"], l, "bk")
                        bv_sb = None
                        if v_bias:
                            bv_sb = cp.tile([1, D], f32, tag="bv", bufs=2, name="bv")
                            nc.sync.dma_start(out=bv_sb, in_=w_in["ca_bv"][l % L:l % L + 1, :])
                        kv_proj(cp, enc_chunk, wk_sb, bk_sb, wv_sb, bv_sb, kT_t, vaug_t)
                        wq_sb = load_w(cp, w_in["ca_wq"], l, D)
                        bq_sb = load_b(cp, w_in["ca_bq"], l, "bq")
                        q_proj(cp, x1T, wq_sb, bq_sb, qT_t)
                        attention(cp, pp2, kT_t, vaug_t, qT_t, oT_t, causal=False)
                        wo_sb = load_w(cp, w_in["ca_wo"], l, D, dt=bf16)
                        bo_sb = load_b(cp, w_in["ca_bo"], l, "bo")
                        out_proj(cp, oT_t, wo_sb, bo_sb,
                                 lambda ft, t: hT[:, ft, t * TCH:(t + 1) * TCH], u2)
                        g2 = load_b(cp, w_in["ln2_g"], l, "g2") if ln_affine else None
                        b2l = load_b(cp, w_in["ln2_b"], l, "b2l") if ln_affine else None
                        layernorm(lp, rows, u2, lambda kc, t: yT[:, kc, t * TCH:(t + 1) * TCH], g2, b2l)

                    # ---- FFN ----
                    fctx = contextlib.ExitStack()
                    with fctx:
                        fp = fctx.enter_context(tc.tile_pool(name=f"ffn{l}", bufs=1))
                        w1_sb = load_w(fp, w_in["ff_w1"], l, F, dt=bf16, tag="w1", bufs=1)
                        b1_sb = load_b(fp, w_in["ff_b1"], l, "b1")
                        w2_sb = fp.tile([P, NFT, D], bf16, tag="w2", bufs=1, name="w2_sb")
                        for kc in range(NFT):
                            nc.sync.dma_start(out=w2_sb[:, kc, :],
                                              in_=w_in["ff_w2b"][l % L, kc * P:(kc + 1) * P, :])
                        b2_sb = load_b(fp, w_in["ff_b2"], l, "b2")
                        h1 = fp.tile([P, NFT, HALF], bf16, tag="h1", bufs=1, name="h1")
                        u3 = lp.tile([P, KC, HALF], f32r, tag="u", bufs=1, name="u3")

                        for ft in range(NFT):
                            pss = [psA.tile([P, TCH], f32, tag="acc", name="f_ps") for _ in range(2)]
                            for kc in range(KC):
                                for t in range(2):
                                    nc.tensor.matmul(pss[t], w1_sb[:, kc, ft * P:(ft + 1) * P],
                                                     yT[:, kc, t * TCH:(t + 1) * TCH],
                                                     start=(kc == 0), stop=(kc == KC - 1))
                            for t in range(2):
                                nc.scalar.activation(h1[:, ft, t * TCH:(t + 1) * TCH], pss[t],
                                                     AF.Relu, bias=b1_sb[:, ft:ft + 1])
                        for ft in range(KC):
                            pss = [psA.tile([P, TCH], f32, tag="acc", name="g_ps") for _ in range(2)]
                            for kc in range(NFT):
                                for t in range(2):
                                    nc.tensor.matmul(pss[t], w2_sb[:, kc, ft * P:(ft + 1) * P],
                                                     h1[:, kc, t * TCH:(t + 1) * TCH],
                                                     start=(kc == 0), stop=False)
                            for t in range(2):
                                nc.tensor.matmul(pss[t], ident_sb,
                                                 x1T[:, ft, t * TCH:(t + 1) * TCH],
                                                 start=False, stop=True)
                                copy_ps(u3[:, ft, t * TCH:(t + 1) * TCH], pss[t], b2_sb[:, ft:ft + 1], t % 2)
                        g3 = load_b(fp, w_in["ln3_g"], l, "g3") if ln_affine else None
                        b3l = load_b(fp, w_in["ln3_b"], l, "b3l") if ln_affine else None
                        layernorm(lp, rows, u3, lambda kc, t: hT[:, kc, t * TCH:(t + 1) * TCH], g3, b3l)

                    # ---- AllGather layer output within the pair (two half-chunks
                    # so the first peer quarter lands as early as possible) ----
                    if l < NLAYERS - 1:
                        for half in range(2):
                            hsl = slice(half * TCH, (half + 1) * TCH)
                            ccin = dramp.tile([P, KC, TCH], f32r, tag="ccin", bufs=4, name="ccin")
                            ccout = dramp.tile([2 * P, KC, TCH], f32r, tag="ccout", bufs=4,
                                               name="ccout")
                            nc.sync.dma_start(out=ccin, in_=hT[:, :, hsl])
                            nc.gpsimd.collective_compute(
                                "AllGather", ALU.bypass, replica_groups=RG,
                                ins=[ccin.opt()], outs=[ccout.opt()])
                            nc.sync.dma_start(out=hT[:, :, HALF + half * TCH:HALF + (half + 1) * TCH],
                                              in_=ccout[ts(peer, P), :, :])

            nc.sync.dma_start(out=out_p[:, :, :], in_=hT[:, :, 0:HALF].bitcast(f32))

    nc.finalize()
    return nc


_BUILD_CACHE = {}
LAST_RESULTS = None


def _get_nc(ln_affine, v_bias):
    key = (ln_affine, v_bias, NLAYERS, OPT_ACTSET, OPT_GMASK, OPT_RECIP)
    if key not in _BUILD_CACHE:
        _BUILD_CACHE[key] = build(ln_affine, v_bias)
    return _BUILD_CACHE[key]


def _to_T(a):  # [S, D] -> [P, KC, S] feature-major
    return np.ascontiguousarray(a.T.reshape(KC, P, S).transpose(1, 0, 2))


def prepare(inputs):
    """Returns (nc, in_maps) for the given full inputs."""
    inp = {k: np.asarray(v) for k, v in inputs.items()}

    ln_affine = not all(
        np.all(inp[f"ln{i}_g"] == 1.0) and np.all(inp[f"ln{i}_b"] == 0.0) for i in (1, 2, 3)
    )
    v_bias = not (np.all(inp["sa_bv"] == 0.0) and np.all(inp["ca_bv"] == 0.0))
    nc = _get_nc(ln_affine, v_bias)

    ones = np.ones((P, P), np.float32)
    ident = np.eye(P, dtype=np.float32)
    pcol = np.arange(P)[:, None]
    qcol = np.arange(TCH)[None, :]
    dmask = np.stack(
        [(qcol >= i * P + pcol) for i in range(4)], axis=1
    ).astype(ml_dtypes.bfloat16)  # [P, 4, TCH]

    shared = {}
    for pre in ("sa", "ca"):
        for nm in ("wq", "wk", "wv", "bq", "bk", "bv", "bo"):
            shared[f"{pre}_{nm}"] = np.ascontiguousarray(inp[f"{pre}_{nm}"], np.float32)
        shared[f"{pre}_wo"] = inp[f"{pre}_wo"].astype(ml_dtypes.bfloat16)
    shared["ff_w1"] = inp["ff_w1"].astype(ml_dtypes.bfloat16)
    shared["ff_b1"] = np.ascontiguousarray(inp["ff_b1"], np.float32)
    shared["ff_w2b"] = inp["ff_w2"].astype(ml_dtypes.bfloat16)
    shared["ff_b2"] = np.ascontiguousarray(inp["ff_b2"], np.float32)
    if ln_affine:
        for i in (1, 2, 3):
            shared[f"ln{i}_g"] = np.ascontiguousarray(inp[f"ln{i}_g"], np.float32)
            shared[f"ln{i}_b"] = np.ascontiguousarray(inp[f"ln{i}_b"], np.float32)
    shared["ones"] = ones
    shared["ident"] = ident
    shared["dmask"] = dmask

    in_maps = []
    for r in range(8):
        b, g = r // 2, r % 2
        mine = [0, 3] if g == 0 else [1, 2]
        theirs = [1, 2] if g == 0 else [0, 3]
        perm = mine + theirs
        xt = np.concatenate([inp["x"][b].T[:, c * TCH:(c + 1) * TCH] for c in perm], axis=1)
        m = dict(shared)
        m["xT"] = np.ascontiguousarray(xt.reshape(KC, P, S).transpose(1, 0, 2))
        m["encT"] = _to_T(np.asarray(inp["enc"][b], np.float32))
        pb = np.zeros(12, np.float32)
        # exp-bias columns: j0 kt8-11 -> 0..3 ; j1 kt8-11 -> 4..7 ; j1 kt12-15 -> 8..11
        # Each group of 4 k-tiles lies in one peer global chunk kg; keep iff kg < qg.
        for base, j, kg in ((0, 0, theirs[0]), (4, 1, theirs[0]), (8, 1, theirs[1])):
            pb[base:base + 4] = 0.0 if kg < mine[j] else NEG
        m["pbias"] = np.broadcast_to(pb, (P, 12)).astype(np.float32).copy()
        in_maps.append(m)
    return nc, in_maps


def unshard(results):
    out = np.zeros((B, S, D), np.float32)
    for r in range(8):
        b, g = r // 2, r % 2
        mine = [0, 3] if g == 0 else [1, 2]
        half = results[r]["out"].transpose(1, 0, 2).reshape(D, HALF)
        for j, c in enumerate(mine):
            out[b, c * TCH:(c + 1) * TCH, :] = half[:, j * TCH:(j + 1) * TCH].T
    return out


def kernel(**inputs):
    global LAST_RESULTS
    nc, in_maps = prepare(inputs)

    res = None
    for attempt in range(3):
        try:
            res = run_bass_kernel_spmd(
                nc, in_maps, core_ids=list(range(8)),
                trace=bool(int(os.environ.get("KERNEL_TRACE", "0"))),
            )
            break
        except Exception:
            # first execution after a fresh NEFF compile occasionally flakes
            # on the runtime side; the NEFF cache makes the retry cheap
            if attempt == 2:
                raise
    LAST_RESULTS = res
    return unshard(res.results)



# revision 6
# speedup vs baseline: 1.6127x; 1.6127x over previous
"""Trainium2 Bass kernel: 6-layer decoder (masked self-attn + cross-attn + FFN).

Sharding (8 cores): 4 batch pairs x 2-way sequence-parallel.
Core r: batch r//2, half g=r%2. Global 512-token chunks: g=0 owns [c0,c3],
g=1 owns [c1,c2] (zigzag for causal load balance). Activations are stored
core-locally as [my 1024 tokens | peer 1024 tokens]; causal structure is
identical across cores (union schedule), with per-core differences expressed
purely as data (exp-bias columns and diagonal mask constants).
One AllGather per layer (within each pair) exchanges the layer output.

Layout: feature-major ("T") everywhere: actT[feat, tok]. All matmuls bf16
(fp32 PSUM accumulation); LN statistics in fp32.

v2 vs baseline:
 - scores for head pairs (2h, 2h+1) issued as two row-tiled matmuls
   (K=64 at partition bases 0/64) into one [128, 1024] PSUM pair tile
   -> concurrent on the PE's 32x32 subarrays, half the score slots.
 - exp batched at N=1024 per pair tile (amortizes ACT's 352-cycle cost).
 - softmax normalization deferred and batched per j-half: AV outputs are
   evicted (values+sums) per head; one Ln + one Exp over all 8 sum rows,
   then 8 broadcast matmuls + muls -> no PE head-of-line blocking.
 - j-outer layer pipeline with "filler" matmul groups (CA kv/q proj, FFN w1)
   interleaved into attention's ACT-bound stretches to keep HAM warm.
 - bf16 everywhere (incl. residual stream) to fit both attentions' K/V and
   the FFN in SBUF simultaneously.
"""

import os

import numpy as np
import ml_dtypes

import concourse.bass as bass
import concourse.mybir as mybir
import concourse.tile as tile
from concourse import bacc
from concourse.bass import ts
from concourse.bass_utils import run_bass_kernel_spmd

L, B, S, D, H, DK, F = 6, 4, 2048, 512, 8, 64, 2048
P = 128
TCH = 512                 # token chunk = matmul free dim
HALF = S // 2             # tokens owned per core
KC = D // P               # 4 partition chunks of d_model
NFT = F // P              # 16 feature tiles of FFN hidden
NKT = S // P              # 16 k-tiles over full sequence
AVW = DK + 1              # V columns per head + ones column (softmax sum)
EPS = 1e-5
SCALE = 1.0 / float(np.sqrt(DK))
NEG = -1e9

f32 = mybir.dt.float32
f32r = mybir.dt.float32r
bf16 = mybir.dt.bfloat16
AF = mybir.ActivationFunctionType
ALU = mybir.AluOpType

NLAYERS = int(os.environ.get("KERNEL_NLAYERS", str(L)))
OPT_ACTSET = bool(int(os.environ.get("KOPT_ACTSET", "1")))
RG = [[0, 1], [2, 3], [4, 5], [6, 7]]

SA_KT = {0: [0, 1, 2, 3, 8, 9, 10, 11], 1: list(range(16))}
CA_KT = {0: list(range(16)), 1: list(range(16))}


def _diag_shape(j, kt):
    if j == 0 and kt < 4:
        return kt
    if j == 1 and 4 <= kt < 8:
        return kt - 4
    return None


def _bias_col(j, kt):
    if kt >= 8:
        return (kt - 8) if j == 0 else 4 + (kt - 8)
    return None


def _single_act_set():
    real = bacc.get_activation_tables

    def patched(arch):
        tabs = real(arch)
        return {name: (fns if name == "natural_log_exp_and_others" else set())
                for name, fns in tabs.items()}

    bacc.get_activation_tables = patched


if OPT_ACTSET:
    _single_act_set()


def build(ln_affine: bool, v_bias: bool):
    nc = bacc.Bacc(None, target_bir_lowering=False, num_devices=8)

    xT = nc.declare_dram_parameter("xT", [P, KC, S], bf16, isOutput=False)
    encT = nc.declare_dram_parameter("encT", [P, KC, S], bf16, isOutput=False)
    w_in = {}
    for pre in ("sa", "ca"):
        for nm in ("wq", "wk", "wv", "wo"):
            w_in[f"{pre}_{nm}"] = nc.declare_dram_parameter(f"{pre}_{nm}", [L, D, D], bf16, isOutput=False)
        for nm in ("bq", "bk", "bo"):
            w_in[f"{pre}_{nm}"] = nc.declare_dram_parameter(f"{pre}_{nm}", [L, D], f32, isOutput=False)
        w_in[f"{pre}_bv"] = nc.declare_dram_parameter(f"{pre}_bv", [L, D], bf16, isOutput=False)
    w_in["ff_w1"] = nc.declare_dram_parameter("ff_w1", [L, D, F], bf16, isOutput=False)
    w_in["ff_b1"] = nc.declare_dram_parameter("ff_b1", [L, F], f32, isOutput=False)
    w_in["ff_w2b"] = nc.declare_dram_parameter("ff_w2b", [L, F, D], bf16, isOutput=False)
    w_in["ff_b2"] = nc.declare_dram_parameter("ff_b2", [L, D], f32, isOutput=False)
    if ln_affine:
        for i in (1, 2, 3):
            w_in[f"ln{i}_g"] = nc.declare_dram_parameter(f"ln{i}_g", [L, D], f32, isOutput=False)
            w_in[f"ln{i}_b"] = nc.declare_dram_parameter(f"ln{i}_b", [L, D], f32, isOutput=False)
    identb_in = nc.declare_dram_parameter("identb", [P, P], bf16, isOutput=False)
    dmask_in = nc.declare_dram_parameter("dmask", [P, 4, 2 * TCH], bf16, isOutput=False)
    pbias_in = nc.declare_dram_parameter("pbias", [P, 12], f32, isOutput=False)
    out_p = nc.declare_dram_parameter("out", [P, KC, HALF], f32, isOutput=True)

    with tile.TileContext(nc, num_cores=8) as tc:
        import contextlib

        gctx = contextlib.ExitStack()
        with gctx:
            persist = gctx.enter_context(tc.tile_pool(name="persist", bufs=1))
            psA = gctx.enter_context(tc.tile_pool(name="psA", bufs=2, space="PSUM"))
            psS = gctx.enter_context(tc.tile_pool(name="psS", bufs=2, space="PSUM"))
            psO = gctx.enter_context(tc.tile_pool(name="psO", bufs=2, space="PSUM"))
            dramp = gctx.enter_context(tc.tile_pool(name="dramp", bufs=2, space="DRAM"))

            hT = persist.tile([P, KC, S], bf16, name="hT")
            ones_bf = persist.tile([P, P], bf16, name="ones_bf")
            ident_sb = persist.tile([P, P], bf16, name="ident_sb")
            dmask_sb = persist.tile([P, 4, 2 * TCH], bf16, name="dmask_sb")
            pbias_sb = persist.tile([P, 12], f32, name="pbias_sb")
            zero_sb = persist.tile([P, 1], f32, name="zero_sb")
            eps_sb = persist.tile([P, 1], f32, name="eps_sb")
            nc.vector.memset(ones_bf, 1.0)
            nc.vector.memset(zero_sb, 0.0)
            nc.vector.memset(eps_sb, EPS)

            for kc in range(KC):
                nc.sync.dma_start(out=hT[:, kc, :], in_=xT[:, kc, :])
            nc.sync.dma_start(out=ident_sb, in_=identb_in[:, :])
            nc.sync.dma_start(out=dmask_sb, in_=dmask_in[:, :, :])
            nc.sync.dma_start(out=pbias_sb, in_=pbias_in[:, :])

            pid = nc.sync.partition_id()
            peer = (pid + 1) % 2

            for l in range(NLAYERS):
                lctx = contextlib.ExitStack()
                with lctx:
                    lp = lctx.enter_context(tc.tile_pool(name=f"lay{l}", bufs=1))
                    rows = lctx.enter_context(tc.tile_pool(name=f"rows{l}", bufs=1))
                    pp = lctx.enter_context(tc.tile_pool(name=f"pt{l}", bufs=1))

                    def load_w(dram_t, cols, tag, bufs=1):
                        n = dram_t.shape[1] // P
                        t = lp.tile([P, n, cols], bf16, tag=tag, bufs=bufs, name=tag)
                        for kc in range(n):
                            nc.sync.dma_start(out=t[:, kc, :],
                                              in_=dram_t[l % L, kc * P:(kc + 1) * P, :])
                        return t

                    def load_b(dram_t, tag):
                        n = dram_t.shape[1] // P
                        t = lp.tile([P, n], f32, tag=tag, bufs=2, name=tag)
                        nc.sync.dma_start(out=t, in_=dram_t[l % L].rearrange("(c p) -> p c", p=P))
                        return t

                    def copy_ps(dst, src_ps, bias_ap, eng):
                        if eng % 2 == 0:
                            nc.vector.tensor_scalar(dst, src_ps, bias_ap, None, ALU.add)
                        else:
                            nc.scalar.activation(dst, src_ps, AF.Identity, bias=bias_ap)

                    # ---- filler machinery ----
                    fillers = []

                    def drain_fillers(n):
                        cnt = 0
                        while fillers and cnt < n:
                            fillers.pop(0)()
                            cnt += 1

                    def drain_all_fillers():
                        while fillers:
                            fillers.pop(0)()

                    enc_cache = {}

                    def enc_chunk(t):
                        if t not in enc_cache:
                            ec = lp.tile([P, KC, TCH], bf16, tag="enc", bufs=2, name="encC")
                            for kc in range(KC):
                                nc.sync.dma_start(out=ec[:, kc, :],
                                                  in_=encT[:, kc, t * TCH:(t + 1) * TCH])
                            enc_cache[t] = ec
                        return enc_cache[t]

                    def own_chunk(t):
                        return hT[:, :, t * TCH:(t + 1) * TCH]

                    def k_group(src_getter, wk_sb, bk_sb, kT_t, t, ft):
                        def go():
                            src = src_getter(t)
                            k_ps = psA.tile([P, TCH], f32, tag="acc", name="k_ps")
                            for kc in range(KC):
                                nc.tensor.matmul(k_ps, wk_sb[:, kc, ft * P:(ft + 1) * P],
                                                 src[:, kc, :], start=(kc == 0), stop=(kc == KC - 1))
                            copy_ps(kT_t[:, ft, t * TCH:(t + 1) * TCH], k_ps,
                                    bk_sb[:, ft:ft + 1], (t + ft) % 2)
                        return go

                    def v_group(src_getter, wv_sb, bv_sb, vaug_t, t, tl):
                        def go():
                            src = src_getter(t)
                            tt = t * 4 + tl
                            v_ps = psA.tile([P, D], f32, tag="acc", name="v_ps")
                            for kc in range(KC):
                                nc.tensor.matmul(v_ps, src[:, kc, tl * P:(tl + 1) * P],
                                                 wv_sb[:, kc, :], start=(kc == 0),
                                                 stop=(kc == KC - 1 and not v_bias))
                            if v_bias:
                                nc.tensor.matmul(v_ps, ones_bf[0:1, :], bv_sb,
                                                 start=False, stop=True)
                            dst = vaug_t[:, tt, 0:H * AVW]
                            dst_ap = bass.AP(tensor=dst.tensor, offset=dst.offset,
                                             ap=[list(dst.ap[0])] + [[AVW, H], [1, DK]])
                            src_ap = bass.AP(tensor=v_ps.tensor, offset=v_ps.offset,
                                             ap=[list(v_ps.ap[0])] + [[DK, H], [1, DK]])
                            nc.vector.tensor_copy(out=dst_ap, in_=src_ap)
                        return go

                    def q_group(srcT, wq_sb, bq_sb, qT_t, ft):
                        def go():
                            q_ps = psA.tile([P, TCH], f32, tag="acc", name="q_ps")
                            for kc in range(KC):
                                nc.tensor.matmul(q_ps, wq_sb[:, kc, ft * P:(ft + 1) * P],
                                                 srcT[:, kc, :],
                                                 start=(kc == 0), stop=(kc == KC - 1))
                            copy_ps(qT_t[:, ft, :], q_ps, bq_sb[:, ft:ft + 1], ft % 2)
                        return go

                    def ffn1_group(yT, w1_sb, b1_sb, h1_t, t, ft):
                        def go():
                            f_ps = psA.tile([P, TCH], f32, tag="acc", name="f_ps")
                            for kc in range(KC):
                                nc.tensor.matmul(f_ps, w1_sb[:, kc, ft * P:(ft + 1) * P],
                                                 yT[:, kc, t * TCH:(t + 1) * TCH],
                                                 start=(kc == 0), stop=(kc == KC - 1))
                            nc.scalar.activation(h1_t[:, ft, :], f_ps,
                                                 AF.Relu, bias=b1_sb[:, ft:ft + 1])
                        return go

                    def init_vaug(vaug_t):
                        ap = bass.AP(tensor=vaug_t.tensor, offset=vaug_t.offset + DK,
                                     ap=[list(vaug_t.ap[0])] + [[H * AVW, NKT], [AVW, H], [1, 1]])
                        nc.vector.memset(ap, 1.0)

                    def attention_j(kT_t, vaug_t, qT_t, oT_t, j, causal, fpc):
                        kts = (SA_KT if causal else CA_KT)[j]
                        CH = 3
                        chunks = [kts[i:i + CH] for i in range(0, len(kts), CH)]
                        nkt = len(kts)
                        ou_t = lp.tile([AVW, H, TCH], bf16, tag="ou", bufs=1, name="ou_t")
                        for hp in range(H // 2):
                            o_ps = [psO.tile([AVW, TCH], f32, tag="oacc", name="o_ps")
                                    for _ in range(2)]
                            done = 0
                            pts = []
                            for ci, ch in enumerate(chunks):
                                pt = pp.tile([P, CH, 2 * TCH], bf16, tag="pt", bufs=2, name="pt")
                                pts.append(pt)
                                for idx, kt in enumerate(ch):
                                    s_ps = psS.tile([P, 2 * TCH], f32, tag="sc", name="s_ps")
                                    for hh in range(2):
                                        nc.tensor.matmul(
                                            s_ps[:, hh * TCH:(hh + 1) * TCH],
                                            kT_t[hh * DK:(hh + 1) * DK, hp, kt * P:(kt + 1) * P],
                                            qT_t[hh * DK:(hh + 1) * DK, hp, :],
                                            start=True, stop=True)
                                    bias = zero_sb[:, 0:1]
                                    if causal:
                                        c = _bias_col(j, kt)
                                        if c is not None:
                                            bias = pbias_sb[:, c:c + 1]
                                    nc.scalar.activation(pt[:, idx, :], s_ps, AF.Exp,
                                                         bias=bias, scale=SCALE)
                                    if causal:
                                        dsh = _diag_shape(j, kt)
                                        if dsh is not None:
                                            nc.gpsimd.tensor_mul(pt[:, idx, :], pt[:, idx, :],
                                                                 dmask_sb[:, dsh, :])
                                drain_fillers(fpc)
                                if ci > 0:
                                    pprev = pts[ci - 1]
                                    for idx in range(CH):
                                        kt = chunks[ci - 1][idx]
                                        for hh in range(2):
                                            nc.tensor.matmul(
                                                o_ps[hh],
                                                vaug_t[:, kt, (2 * hp + hh) * AVW:(2 * hp + hh + 1) * AVW],
                                                pprev[:, idx, hh * TCH:(hh + 1) * TCH],
                                                start=(done == 0), stop=False)
                                        done += 1
                            pprev = pts[-1]
                            for idx in range(len(chunks[-1])):
                                kt = chunks[-1][idx]
                                for hh in range(2):
                                    nc.tensor.matmul(
                                        o_ps[hh],
                                        vaug_t[:, kt, (2 * hp + hh) * AVW:(2 * hp + hh + 1) * AVW],
                                        pprev[:, idx, hh * TCH:(hh + 1) * TCH],
                                        start=(done == 0), stop=(done == nkt - 1))
                                done += 1
                            for hh in range(2):
                                nc.vector.tensor_copy(out=ou_t[:, 2 * hp + hh, :], in_=o_ps[hh])
                        # batched reciprocal of all 8 sum rows (partition 64)
                        sr = ou_t[DK:AVW, :, :]
                        nc.scalar.activation(sr, sr, AF.Ln, bias=zero_sb[DK:DK + 1, 0:1])
                        nc.scalar.activation(sr, sr, AF.Exp, scale=-1.0,
                                             bias=zero_sb[DK:DK + 1, 0:1])
                        for h in range(H):
                            r_ps = psA.tile([DK, TCH], f32, tag="acc", name="r_ps")
                            nc.tensor.matmul(r_ps, ones_bf[DK:DK + 1, 0:DK],
                                             ou_t[DK:AVW, h, :], start=True, stop=True)
                            nc.vector.tensor_mul(
                                oT_t[(h % 2) * DK:(h % 2) * DK + DK, h // 2, :],
                                ou_t[0:DK, h, :], r_ps)

                    def out_proj_j(oT_t, wo_sb, bo_sb, resT, u_t, t):
                        for ft in range(KC):
                            ps = psA.tile([P, TCH], f32, tag="acc", name="u_ps")
                            for kc in range(KC):
                                nc.tensor.matmul(ps, wo_sb[:, kc, ft * P:(ft + 1) * P],
                                                 oT_t[:, kc, :],
                                                 start=(kc == 0), stop=False)
                            nc.tensor.matmul(ps, ident_sb, resT(ft, t), start=False, stop=True)
                            copy_ps(u_t[:, ft, t * TCH:(t + 1) * TCH], ps,
                                    bo_sb[:, ft:ft + 1], ft % 2)

                    def layernorm_t(u_t, dst, g_sb, b_sb, t):
                        tsl = slice(t * TCH, (t + 1) * TCH)
                        usq = lp.tile([P, KC, TCH], bf16, tag="usq", bufs=1, name="usq")
                        for kc in range(KC):
                            nc.vector.tensor_mul(usq[:, kc, :], u_t[:, kc, tsl], u_t[:, kc, tsl])
                        m_ps = psA.tile([P, TCH], f32, tag="acc", name="m_ps")
                        for kc in range(KC):
                            nc.tensor.matmul(m_ps, ones_bf, u_t[:, kc, tsl],
                                             start=(kc == 0), stop=(kc == KC - 1))
                        q_ps = psA.tile([P, TCH], f32, tag="acc", name="q_ps")
                        for kc in range(KC):
                            nc.tensor.matmul(q_ps, ones_bf, usq[:, kc, :],
                                             start=(kc == 0), stop=(kc == KC - 1))
                        t_sb = rows.tile([P, TCH], f32, tag="rows", bufs=4, name="t_sb")
                        nc.vector.tensor_scalar(t_sb, m_ps, 1.0 / D, None, ALU.mult)
                        m2 = rows.tile([P, TCH], f32, tag="rows", bufs=4, name="m2")
                        nc.vector.tensor_scalar(m2, q_ps, 1.0 / D, None, ALU.mult)
                        tt2 = rows.tile([P, TCH], f32, tag="rows", bufs=4, name="tt2")
                        nc.vector.tensor_mul(tt2, t_sb, t_sb)
                        nc.vector.tensor_sub(m2, m2, tt2)
                        nc.scalar.activation(m2, m2, AF.Ln, bias=eps_sb[:, 0:1])
                        r_sb = rows.tile([P, TCH], f32, tag="rows", bufs=4, name="r_sb")
                        nc.scalar.activation(r_sb, m2, AF.Exp, scale=-0.5, bias=zero_sb[:, 0:1])
                        c_sb = rows.tile([P, TCH], f32, tag="rows", bufs=4, name="c_sb")
                        nc.vector.tensor_mul(c_sb, t_sb, r_sb)
                        for kc in range(KC):
                            tmp = rows.tile([P, TCH], f32, tag="ltmp", bufs=2, name="ltmp")
                            nc.vector.tensor_sub(tmp, u_t[:, kc, tsl], c_sb)
                            d = dst(kc, t)
                            nc.vector.tensor_mul(d, tmp, r_sb)
                            if ln_affine:
                                nc.vector.tensor_scalar(d, d, g_sb[:, kc:kc + 1],
                                                        b_sb[:, kc:kc + 1], ALU.mult, ALU.add)

                    # ---- per-layer tensors ----
                    x1T = lp.tile([P, KC, HALF], bf16, tag="x1", name="x1T")
                    yT = lp.tile([P, KC, HALF], bf16, tag="y", name="yT")
                    sa_kT = lp.tile([P, KC, S], bf16, tag="kv", name="kT_sa")
                    sa_va = lp.tile([P, NKT, H * AVW], bf16, tag="va", name="va_sa")
                    ca_kT = lp.tile([P, KC, S], bf16, tag="kvc", name="kT_ca")
                    ca_va = lp.tile([P, NKT, H * AVW], bf16, tag="vac", name="va_ca")
                    u1 = lp.tile([P, KC, HALF], bf16, tag="u", name="u1")

                    # ---- weight loads (ring tenancy order matters) ----
                    sa_wk = load_w(w_in["sa_wk"], D, tag="wkv", bufs=2)
                    sa_wv = load_w(w_in["sa_wv"], D, tag="wkv", bufs=2)
                    sa_bk = load_b(w_in["sa_bk"], "sabk")
                    sa_bv = None
                    ca_bv = None
                    if v_bias:
                        sa_bv = lp.tile([1, D], bf16, tag="bv", bufs=2, name="sabv")
                        nc.sync.dma_start(out=sa_bv, in_=w_in["sa_bv"][l % L:l % L + 1, :])
                        ca_bv = lp.tile([1, D], bf16, tag="bv", bufs=2, name="cabv")
                        nc.sync.dma_start(out=ca_bv, in_=w_in["ca_bv"][l % L:l % L + 1, :])
                    sa_wq = load_w(w_in["sa_wq"], D, tag="wq2", bufs=2)
                    sa_bq = load_b(w_in["sa_bq"], "sabq")
                    ca_wq = load_w(w_in["ca_wq"], D, tag="wq2", bufs=2)
                    ca_bq = load_b(w_in["ca_bq"], "cabq")
                    ca_wk = load_w(w_in["ca_wk"], D, tag="wkv", bufs=2)
                    ca_wv = load_w(w_in["ca_wv"], D, tag="wkv", bufs=2)
                    ca_bk = load_b(w_in["ca_bk"], "cabk")

                    init_vaug(sa_va)
                    init_vaug(ca_va)

                    # ---- SA K/V own tokens + Q (dense PE warmup) ----
                    sa_q = []
                    for t in (0, 1):
                        for ft in range(KC):
                            k_group(own_chunk, sa_wk, sa_bk, sa_kT, t, ft)()
                        for tl in range(4):
                            v_group(own_chunk, sa_wv, sa_bv, sa_va, t, tl)()
                    for jq in (0, 1):
                        qt = lp.tile([P, KC, TCH], bf16, tag="qT", bufs=2, name=f"saq{jq}")
                        sa_q.append(qt)
                        for ft in range(KC):
                            q_group(hT[:, :, jq * TCH:(jq + 1) * TCH], sa_wq, sa_bq, qt, ft)()
                    # SA peer-token K/V (depends on prev layer's AllGather)
                    for t in (2, 3):
                        for ft in range(KC):
                            k_group(own_chunk, sa_wk, sa_bk, sa_kT, t, ft)()
                        for tl in range(4):
                            v_group(own_chunk, sa_wv, sa_bv, sa_va, t, tl)()

                    # queue CA kv as fillers for SA attention
                    for t in range(4):
                        for ft in range(KC):
                            fillers.append(k_group(enc_chunk, ca_wk, ca_bk, ca_kT, t, ft))
                        for tl in range(4):
                            fillers.append(v_group(enc_chunk, ca_wv, ca_bv, ca_va, t, tl))

                    sa_wo = load_w(w_in["sa_wo"], D, tag="wq2", bufs=2)
                    sa_bo = load_b(w_in["sa_bo"], "sabo")
                    g1 = load_b(w_in["ln1_g"], "g1") if ln_affine else None
                    b1l = load_b(w_in["ln1_b"], "b1l") if ln_affine else None
                    ca_wo = load_w(w_in["ca_wo"], D, tag="wq2", bufs=2)
                    ca_bo = load_b(w_in["ca_bo"], "cabo")
                    g2 = load_b(w_in["ln2_g"], "g2") if ln_affine else None
                    b2l = load_b(w_in["ln2_b"], "b2l") if ln_affine else None

                    # ---- SA attention / out / LN1 / CA q_proj, per j ----
                    ca_q = []
                    for j in (0, 1):
                        oT = lp.tile([P, KC, TCH], bf16, tag="oT", bufs=2, name=f"saoT{j}")
                        attention_j(sa_kT, sa_va, sa_q[j], oT, j, True, fpc=2)
                        out_proj_j(oT, sa_wo, sa_bo,
                                   lambda ft, t: hT[:, ft, t * TCH:(t + 1) * TCH], u1, j)
                        layernorm_t(u1, lambda kc, t: x1T[:, kc, t * TCH:(t + 1) * TCH],
                                    g1, b1l, j)
                        qt = lp.tile([P, KC, TCH], bf16, tag="qT", bufs=2, name=f"caq{j}")
                        ca_q.append(qt)
                        for ft in range(KC):
                            fillers.append(q_group(x1T[:, :, j * TCH:(j + 1) * TCH],
                                                   ca_wq, ca_bq, qt, ft))
                    drain_all_fillers()

                    # w1 shares sa_va's region (freed after SA attention)
                    w1_sb = load_w(w_in["ff_w1"], F, tag="va", bufs=1)
                    b1_sb = load_b(w_in["ff_b1"], "b1")
                    u2 = lp.tile([P, KC, HALF], bf16, tag="u", name="u2")

                    # ---- CA attention / out / LN2, per j (FFN w1 j0 as fillers
                    # for CA attention j1) ----
                    b2_sb = load_b(w_in["ff_b2"], "b2")
                    g3 = load_b(w_in["ln3_g"], "g3") if ln_affine else None
                    b3l = load_b(w_in["ln3_b"], "b3l") if ln_affine else None
                    last = l == NLAYERS - 1
                    h1s = []
                    for j in (0, 1):
                        oT = lp.tile([P, KC, TCH], bf16, tag="oT", bufs=2, name=f"caoT{j}")
                        attention_j(ca_kT, ca_va, ca_q[j], oT, j, False, fpc=2)
                        out_proj_j(oT, ca_wo, ca_bo,
                                   lambda ft, t: hT[:, ft, t * TCH:(t + 1) * TCH], u2, j)
                        layernorm_t(u2, lambda kc, t: yT[:, kc, t * TCH:(t + 1) * TCH],
                                    g2, b2l, j)
                        h1_t = lp.tile([P, NFT, TCH], bf16, tag="h1", bufs=1, name=f"h1_{j}")
                        h1s.append(h1_t)
                        if j == 0:
                            for ft in range(NFT):
                                fillers.append(ffn1_group(yT, w1_sb, b1_sb, h1_t, j, ft))
                        else:
                            drain_all_fillers()
                    u3 = lp.tile([P, KC, HALF], bf16, tag="u", name="u3")
                    if last:
                        outT = lp.tile([P, KC, HALF], f32, tag="kvc", name="outT")

                    # w2 shares sa_kT's region (freed after SA attention)
                    w2_sb = lp.tile([P, NFT, D], bf16, tag="kv", name="w2_sb")
                    for kc in range(NFT):
                        nc.sync.dma_start(out=w2_sb[:, kc, :],
                                          in_=w_in["ff_w2b"][l % L, kc * P:(kc + 1) * P, :])

                    # ---- FFN w2(j) + LN3(j) + AllGather(j); w1(j1) emitted
                    # between the two j halves (h1 ring: w2 j0 must precede) ----
                    def ffn2_ln3(j):
                        for ft in range(KC):
                            ps = psA.tile([P, TCH], f32, tag="acc", name="g_ps")
                            for kc in range(NFT):
                                nc.tensor.matmul(ps, w2_sb[:, kc, ft * P:(ft + 1) * P],
                                                 h1s[j][:, kc, :],
                                                 start=(kc == 0), stop=False)
                            nc.tensor.matmul(ps, ident_sb,
                                             x1T[:, ft, j * TCH:(j + 1) * TCH],
                                             start=False, stop=True)
                            copy_ps(u3[:, ft, j * TCH:(j + 1) * TCH], ps,
                                    b2_sb[:, ft:ft + 1], ft % 2)
                        if last:
                            layernorm_t(u3, lambda kc, t: outT[:, kc, t * TCH:(t + 1) * TCH],
                                        g3, b3l, j)
                        else:
                            layernorm_t(u3, lambda kc, t: hT[:, kc, t * TCH:(t + 1) * TCH],
                                        g3, b3l, j)
                            hsl = slice(j * TCH, (j + 1) * TCH)
                            ccin = dramp.tile([P, KC, TCH], bf16, tag="ccin", bufs=4, name="ccin")
                            ccout = dramp.tile([2 * P, KC, TCH], bf16, tag="ccout", bufs=4,
                                               name="ccout")
                            nc.sync.dma_start(out=ccin, in_=hT[:, :, hsl])
                            nc.gpsimd.collective_compute(
                                "AllGather", ALU.bypass, replica_groups=RG,
                                ins=[ccin.opt()], outs=[ccout.opt()])
                            nc.sync.dma_start(out=hT[:, :, HALF + j * TCH:HALF + (j + 1) * TCH],
                                              in_=ccout[ts(peer, P), :, :])

                    ffn2_ln3(0)
                    for ft in range(NFT):
                        ffn1_group(yT, w1_sb, b1_sb, h1s[1], 1, ft)()
                    ffn2_ln3(1)

                    if last:
                        nc.sync.dma_start(out=out_p[:, :, :], in_=outT)

    nc.finalize()
    return nc


_BUILD_CACHE = {}
LAST_RESULTS = None


def _get_nc(ln_affine, v_bias):
    key = (ln_affine, v_bias, NLAYERS, OPT_ACTSET)
    if key not in _BUILD_CACHE:
        _BUILD_CACHE[key] = build(ln_affine, v_bias)
    return _BUILD_CACHE[key]


def _to_T(a):  # [S, D] -> [P, KC, S] feature-major, bf16
    return np.ascontiguousarray(
        a.T.reshape(KC, P, S).transpose(1, 0, 2)).astype(ml_dtypes.bfloat16)


def prepare(inputs):
    """Returns (nc, in_maps) for the given full inputs."""
    inp = {k: np.asarray(v) for k, v in inputs.items()}

    ln_affine = not all(
        np.all(inp[f"ln{i}_g"] == 1.0) and np.all(inp[f"ln{i}_b"] == 0.0) for i in (1, 2, 3)
    )
    v_bias = not (np.all(inp["sa_bv"] == 0.0) and np.all(inp["ca_bv"] == 0.0))
    nc = _get_nc(ln_affine, v_bias)

    ident = np.eye(P, dtype=np.float32)
    pcol = np.arange(P)[:, None]
    qcol = np.arange(TCH)[None, :]
    dmask1 = np.stack(
        [(qcol >= i * P + pcol) for i in range(4)], axis=1
    ).astype(ml_dtypes.bfloat16)  # [P, 4, TCH]
    dmask = np.concatenate([dmask1, dmask1], axis=2)  # head-pair duplicated

    shared = {}
    for pre in ("sa", "ca"):
        for nm in ("wq", "wk", "wv", "wo"):
            shared[f"{pre}_{nm}"] = inp[f"{pre}_{nm}"].astype(ml_dtypes.bfloat16)
        for nm in ("bq", "bk", "bo"):
            shared[f"{pre}_{nm}"] = np.ascontiguousarray(inp[f"{pre}_{nm}"], np.float32)
        shared[f"{pre}_bv"] = inp[f"{pre}_bv"].astype(ml_dtypes.bfloat16)
    shared["ff_w1"] = inp["ff_w1"].astype(ml_dtypes.bfloat16)
    shared["ff_b1"] = np.ascontiguousarray(inp["ff_b1"], np.float32)
    shared["ff_w2b"] = inp["ff_w2"].astype(ml_dtypes.bfloat16)
    shared["ff_b2"] = np.ascontiguousarray(inp["ff_b2"], np.float32)
    if ln_affine:
        for i in (1, 2, 3):
            shared[f"ln{i}_g"] = np.ascontiguousarray(inp[f"ln{i}_g"], np.float32)
            shared[f"ln{i}_b"] = np.ascontiguousarray(inp[f"ln{i}_b"], np.float32)
    shared["identb"] = ident.astype(ml_dtypes.bfloat16)
    shared["dmask"] = dmask

    in_maps = []
    for r in range(8):
        b, g = r // 2, r % 2
        mine = [0, 3] if g == 0 else [1, 2]
        theirs = [1, 2] if g == 0 else [0, 3]
        perm = mine + theirs
        xt = np.concatenate([inp["x"][b].T[:, c * TCH:(c + 1) * TCH] for c in perm], axis=1)
        m = dict(shared)
        m["xT"] = np.ascontiguousarray(
            xt.reshape(KC, P, S).transpose(1, 0, 2)).astype(ml_dtypes.bfloat16)
        m["encT"] = _to_T(np.asarray(inp["enc"][b], np.float32))
        pb = np.zeros(12, np.float32)
        # exp-bias columns: j0 kt8-11 -> 0..3 ; j1 kt8-11 -> 4..7 ; j1 kt12-15 -> 8..11
        for base, j, kg in ((0, 0, theirs[0]), (4, 1, theirs[0]), (8, 1, theirs[1])):
            pb[base:base + 4] = 0.0 if kg < mine[j] else NEG
        m["pbias"] = np.broadcast_to(pb, (P, 12)).astype(np.float32).copy()
        in_maps.append(m)
    return nc, in_maps


def unshard(results):
    out = np.zeros((B, S, D), np.float32)
    for r in range(8):
        b, g = r // 2, r % 2
        mine = [0, 3] if g == 0 else [1, 2]
        half = results[r]["out"].transpose(1, 0, 2).reshape(D, HALF)
        for j, c in enumerate(mine):
            out[b, c * TCH:(c + 1) * TCH, :] = half[:, j * TCH:(j + 1) * TCH].T
    return out


def kernel(**inputs):
    global LAST_RESULTS
    nc, in_maps = prepare(inputs)

    res = None
    for attempt in range(3):
        try:
            res = run_bass_kernel_spmd(
                nc, in_maps, core_ids=list(range(8)),
                trace=bool(int(os.environ.get("KERNEL_TRACE", "0"))),
            )
            break
        except Exception:
            # first execution after a fresh NEFF compile occasionally flakes
            # on the runtime side; the NEFF cache makes the retry cheap
            if attempt == 2:
                raise
    LAST_RESULTS = res
    return unshard(res.results)
